# revision 20
# baseline (speedup 1.0000x reference)
"""Multi-head attention (RMSNorm-QK + RoPE + softmax + proj) on 8 Trainium2 cores.

Sharding: core c handles batch b = c//4 and heads [3*(c%4), 3*(c%4)+3).
Each core computes qkv for its heads, flash-style attention, and a partial
projection over its heads' channels; the host sums the 4 partials per batch.

Design notes:
 - all matmul moving operands are bf16 (1 cyc/row on the PE at any free
   size), weights/x/tables DMA'd as bf16 to halve input traffic.
 - PV matmul in flipped [q,d] orientation (px stationary, v moving, 65-row
   outputs incl. a ones-column for the softmax denominator), halving PV cost
   vs the [d,q] orientation.
 - softmax epilogue: DVE reciprocal of the denominator column + per-q-chunk
   tensor_scalar, then a small PE transpose (through scratch space in the o
   PSUM bank) back to [d,q] for the projection.
 - q^T/k^T layout [head_dim, tokens]; head-dim rows permuted so the RoPE
   half-swap is an intra-quadrant stream_shuffle.
 - RMS-norm: sum(q^2) via ones-pair matmul; rsqrt = exp(-0.5*ln(x)); one ACT
   table set for the whole kernel.
 - the attention inner loop is a software-pipelined stream of 96 S->exp->PV
   groups; the S matmuls of group g+1 are emitted before the filler work of
   group g so the in-order PE queue always serves the Act-critical path
   first. qkv for heads 1,2, v-compute, epilogues and projection are diced
   into ~1-2us work items placed into one slot per group.
 - elementwise work is split between DVE (shuffle, squares, adds, epilogue)
   and the Pool/GPSIMD engine (cos-mul, k-scale, PSUM->SBUF copies).
"""
import sys

for _p in ("/opt/trn_rl_repo", "/opt/trn_rl_repo/concourse"):
    if _p not in sys.path:
        sys.path.insert(0, _p)

import numpy as np
from contextlib import ExitStack

import concourse.bass as bass
import concourse.tile as tile
import concourse.mybir as mybir
from concourse.bass_utils import run_bass_kernel_spmd

F32 = mybir.dt.float32
F32R = mybir.dt.float32r
BF16 = mybir.dt.bfloat16
AF = mybir.ActivationFunctionType

B, N, C = 2, 2048, 768
H, HD = 12, 64
HP = 3            # heads per core
NCORES = 8
CCH = C // 128    # 6 contraction chunks
NT = N // 512     # 4 token tiles of 512
KB = N // 128     # 16 k-blocks of 128
EPS = 1e-6

SWAP_MASK = [(i + 16) % 32 for i in range(32)]
# head-dim permutation: pair-exchange (d <-> d+32) becomes intra-quadrant
PERM = np.concatenate([np.arange(0, 16), np.arange(32, 48),
                       np.arange(16, 32), np.arange(48, 64)])
SIGN = np.where(PERM < 32, -1.0, 1.0).astype(np.float32)

_NC_CACHE = {}


def build_nc(split_waits=True):
    nc = bass.Bass(target_bir_lowering=True)
    xT = nc.declare_dram_parameter("xT", [C, N], BF16, isOutput=False)
    # weights packed chunk-horizontal so one DMA covers several chunks
    wqk2 = nc.declare_dram_parameter("wqk2", [128, CCH * HP * 128], BF16,
                                     isOutput=False)
    wv2 = nc.declare_dram_parameter("wv2", [128, CCH * HP * 64], BF16,
                                    isOutput=False)
    bqkbv = nc.declare_dram_parameter("bqkbv", [1, HP * 192], BF16,
                                      isOutput=False)
    cos2w = nc.declare_dram_parameter("cos2w", [128, N], BF16, isOutput=False)
    sinSw = nc.declare_dram_parameter("sinSw", [128, N], BF16, isOutput=False)
    wp = nc.declare_dram_parameter("wp", [HP * HD, C], BF16, isOutput=False)
    identd = nc.declare_dram_parameter("identd", [128, 128], F32R,
                                       isOutput=False)
    out = nc.declare_dram_parameter("out", [N, C], BF16, isOutput=True)

    with tile.TileContext(nc) as tc, ExitStack() as ctx:
        sb = ctx.enter_context(tc.tile_pool(name="sb", bufs=1))
        tp = ctx.enter_context(tc.tile_pool(name="tp", bufs=2))
        pe = ctx.enter_context(tc.tile_pool(name="pe", bufs=3))   # pexp
        tp1 = ctx.enter_context(tc.tile_pool(name="tp1", bufs=2))
        fps = ctx.enter_context(tc.tile_pool(name="fps", bufs=2, space="PSUM"))
        sA = ctx.enter_context(tc.tile_pool(name="sA", bufs=1, space="PSUM"))
        sB = ctx.enter_context(tc.tile_pool(name="sB", bufs=1, space="PSUM"))
        oA = ctx.enter_context(tc.tile_pool(name="oA", bufs=1, space="PSUM"))
        oB = ctx.enter_context(tc.tile_pool(name="oB", bufs=1, space="PSUM"))

        # ---------- prologue ----------
        # x half-chunks alternate the two HWDGE queues (SP/Act) so the qkv
        # matmuls chase the loads; big weight packs ride SWDGE (Pool);
        # small constants are memset-derived to keep the DMA count low
        # (each HWDGE issue serializes ~0.65us on the single HWDGE device).
        # memset-derived constants (emitted first: Pool runs these before
        # its SWDGE issue backlog so the RoPE chain isn't gated on them)
        ones_row = sb.tile([1, 512], BF16, tag="ones_row")
        nc.gpsimd.memset(ones_row[:], 1.0)
        onesp = sb.tile([128, 2], BF16, tag="onesp")
        nc.gpsimd.memset(onesp[:], 0.0)
        nc.gpsimd.memset(onesp[0:64, 0:1], 1.0)
        nc.gpsimd.memset(onesp[64:128, 1:2], 1.0)
        eps_t = sb.tile([128, 1], F32, tag="eps")
        nc.gpsimd.memset(eps_t[:], EPS)
        # v3i: per (head, kb) a [128, 65] block: v columns 0:64, ones col 64
        v3i = sb.tile([128, HP * KB * 65], BF16, tag="v3i")
        nc.gpsimd.memset(
            v3i[:].rearrange("p (b n) -> p b n", n=65)[:, :, 64:65], 1.0)
        s_sb = sb.tile([128, 512], F32, tag="s_sb")
        nc.gpsimd.memset(s_sb[:], 1.0)

        bqkbv_sb = sb.tile([1, HP * 192], BF16, tag="bqkbv")
        nc.gpsimd.dma_start(bqkbv_sb[:], bqkbv[:, :])
        wqk_sb = sb.tile([128, CCH * HP * 128], BF16, tag="wqk")
        nc.gpsimd.dma_start(wqk_sb[:, 0:HP * 384], wqk2[:, 0:HP * 384])
        nc.gpsimd.dma_start(wqk_sb[:, HP * 384:], wqk2[:, HP * 384:])
        cos_sb = sb.tile([128, N], BF16, tag="cos")
        nc.gpsimd.dma_start(cos_sb[:, 0:1024], cos2w[:, 0:1024])
        sin_sb = sb.tile([128, N], BF16, tag="sin")
        nc.gpsimd.dma_start(sin_sb[:, 0:1024], sinSw[:, 0:1024])
        wv_sb = sb.tile([128, CCH * HP * 64], BF16, tag="wv")
        nc.gpsimd.dma_start(wv_sb[:], wv2[:, :])
        nc.gpsimd.dma_start(cos_sb[:, 1024:2048], cos2w[:, 1024:2048])
        nc.gpsimd.dma_start(sin_sb[:, 1024:2048], sinSw[:, 1024:2048])
        xs = []
        for c in range(CCH):
            t = sb.tile([128, N], BF16, tag=f"x{c}")
            nc.sync.dma_start(t[:, 0:1024], xT[c * 128:(c + 1) * 128, 0:1024])
            nc.scalar.dma_start(t[:, 1024:2048],
                                xT[c * 128:(c + 1) * 128, 1024:2048])
            xs.append(t)
        wp0_sb = sb.tile([128, C], BF16, tag="wp0")
        nc.sync.dma_start(wp0_sb[:], wp[0:128, :])
        wp1_sb = sb.tile([64, C], BF16, tag="wp1")
        nc.sync.dma_start(wp1_sb[:], wp[128:192, :])
        ident = sb.tile([128, 128], F32R, tag="ident")
        nc.sync.dma_start(ident[:], identd[:, :])

        def wqk_c(c, h):
            return wqk_sb[:, c * HP * 128 + h * 128:c * HP * 128 + (h + 1) * 128]

        def wv_c(c):
            return wv_sb[:, c * HP * 64:(c + 1) * HP * 64]

        bqk_sb = bqkbv_sb[:, 0:HP * 128]
        bv_sb = bqkbv_sb[:, HP * 128:HP * 192]


        # qT/kT packed by head pairs so S-matmul operands share a base partition
        q12 = sb.tile([128, N], BF16, tag="q12")
        k12 = sb.tile([128, N], BF16, tag="k12")
        q3 = sb.tile([64, N], BF16, tag="q3")
        k3 = sb.tile([64, N], BF16, tag="k3")

        def qT(h):
            return (q12[0:64], q12[64:128], q3[:])[h]

        def kT(h):
            return (k12[0:64], k12[64:128], k3[:])[h]

        oall_a = sb.tile([128, N], BF16, tag="oall_a")   # heads 0,1 O^T
        oall_b = sb.tile([64, N], BF16, tag="oall_b")    # head 2 O^T
        t4_all = sb.tile([128, N], BF16, tag="t4_all")

        def mm(out_ap, lhsT, rhs, start, stop):
            nc.tensor.matmul(out_ap, lhsT, rhs,
                             start=start, stop=stop, skip_group_check=True)

        # ---------- qkv work items (split into matmul and vector halves) ----
        qk_box = {}

        def qkv_passA_mm(h, t):
            ts = slice(t * 512, (t + 1) * 512)
            qk_ps = fps.tile([128, 512], F32, tag="flex")
            for c in range(CCH):
                mm(qk_ps[:], wqk_c(c, h), xs[c][:, ts], c == 0, False)
            mm(qk_ps[:], bqk_sb[:, h * 128:(h + 1) * 128], ones_row[:],
               False, True)
            qk_box[(h, t)] = qk_ps

        def qkv_passA_ve(h, t):
            ts = slice(t * 512, (t + 1) * 512)
            qk_ps = qk_box.pop((h, t))
            t1 = tp1.tile([128, 512], BF16, tag="t1")
            nc.vector.tensor_mul(t1[:], qk_ps[:], cos_sb[:, ts])
            t2 = tp.tile([128, 512], BF16, tag="t2")
            nc.vector.stream_shuffle(t2[:], qk_ps[:], SWAP_MASK)
            sq = tp.tile([128, 512], BF16, tag="sq")
            nc.gpsimd.tensor_mul(sq[:], t2[:], t2[:])
            t3 = tp.tile([128, 512], BF16, tag="t3")
            nc.gpsimd.tensor_mul(t3[:], t2[:], sin_sb[:, ts])
            mm(qk_ps[0:2, :], onesp[:], sq[:], True, True)
            nc.vector.tensor_copy(s_sb[32 * t:32 * t + 2, :], qk_ps[0:2, :])
            nc.gpsimd.tensor_add(t4_all[:, ts], t1[:], t3[:])

        def qkv_passA(h, t):
            qkv_passA_mm(h, t)
            qkv_passA_ve(h, t)

        def qkv_ln(h):
            lnv = tp1.tile([128, 512], F32, tag="lnv")
            nc.scalar.activation(lnv[:], s_sb[:], AF.Ln,
                                 bias=eps_t[:], scale=1.0 / HD)
            sv = tp1.tile([128, 512], BF16, tag="sv")
            nc.scalar.activation(sv[:], lnv[:], AF.Exp, bias=0.0, scale=-0.5)
            return sv

        def qkv_ft(h, t, sv):
            # broadcast the per-token rsqrt rows to 64 partitions via DMA
            # (stride-0 source) so the q/k scale muls run as all-bf16 SBUF
            # DVE ops in 2x mode.
            ts = slice(t * 512, (t + 1) * 512)
            bc = tp.tile([128, 512], BF16, tag="bc")
            nc.sync.dma_start(
                bc[0:64, :],
                sv[32 * t:32 * t + 1, :][:, None, :]
                .broadcast_to((1, 64, 512)))
            nc.sync.dma_start(
                bc[64:128, :],
                sv[32 * t + 1:32 * t + 2, :][:, None, :]
                .broadcast_to((1, 64, 512)))
            nc.vector.tensor_mul(qT(h)[:, ts], t4_all[0:64, ts], bc[0:64, :])
            nc.vector.tensor_mul(kT(h)[:, ts], t4_all[64:128, ts],
                                 bc[64:128, :])

        # ---------- v for all heads, one tt-pair (mm and copy halves) -------
        v_box = {}

        def v_pair_mm(p):
            v_ps = fps.tile([128, 384], F32, tag="flex")
            for i, tt in enumerate((2 * p, 2 * p + 1)):
                for h in range(HP):
                    vs = slice((i * HP + h) * 64, (i * HP + h + 1) * 64)
                    for c in range(CCH):
                        mm(v_ps[:, vs], xs[c][:, tt * 128:(tt + 1) * 128],
                           wv_c(c)[:, h * 64:(h + 1) * 64], c == 0, False)
                    mm(v_ps[:, vs], ones_row[0:1, 0:128],
                       bv_sb[:, h * 64:(h + 1) * 64], False, True)
            v_box[p] = v_ps

        def v_pair_cp(p):
            v_ps = v_box.pop(p)
            dst = v3i[:].rearrange("p (g k n) -> p g k n", g=HP, k=KB)
            nc.vector.tensor_copy(
                dst[:, :, 2 * p:2 * p + 2, 0:64],
                v_ps[:].rearrange("p (i g n) -> p g i n", i=2, g=HP))

        def v_pair(p):
            v_pair_mm(p)
            v_pair_cp(p)

        # ---------- epilogue + projection work items ----------
        def epi_a(h, qt, o_t):
            rec = tp1.tile([128, 4], F32, tag="rec")
            nc.vector.reciprocal_approx_fast(rec[:], o_t[:, 64:64 + 4 * 65:65])
            o_n = tp.tile([128, 256], F32R, tag="o_n")
            for qc in range(4):
                nc.vector.tensor_scalar_mul(
                    o_n[:, qc * 64:(qc + 1) * 64],
                    o_t[:, qc * 65:qc * 65 + 64], rec[:, qc:qc + 1])
            return o_n

        def epi_b(h, qt, o_t, o_n):
            for qc in range(4):
                # transpose [128q, 64d] -> [64d, 128q] via PE, scratch in
                # the unused tail of the o PSUM bank
                nc.tensor.matmul(o_t[0:64, 384:512].bitcast(F32R),
                                 o_n[:, qc * 64:(qc + 1) * 64],
                                 ident[:],
                                 start=True, stop=True, is_transpose=True,
                                 skip_group_check=True)
                cs = slice(qt * 512 + qc * 128, qt * 512 + (qc + 1) * 128)
                dst = oall_a[h * 64:(h + 1) * 64, cs] if h < 2 \
                    else oall_b[:, cs]
                nc.vector.tensor_copy(dst, o_t[0:64, 384:512])

        def proj_tt(tt):
            po = tp.tile([128, C], BF16, tag="po")
            for half in range(2):
                cs = slice(half * 384, (half + 1) * 384)
                p_ps = fps.tile([128, 384], F32, tag="flex")
                mm(p_ps[:], oall_a[:, tt * 128:(tt + 1) * 128],
                   wp0_sb[:, cs], True, False)
                mm(p_ps[:], oall_b[:, tt * 128:(tt + 1) * 128],
                   wp1_sb[:, cs], False, True)
                nc.vector.tensor_copy(po[:, cs], p_ps[:])
            nc.sync.dma_start(out[tt * 128:(tt + 1) * 128, :], po[:])

        # ---------- lead-in: tiles 0/1 of head 0 (x h0-halves only) ----------
        # First pexp only needs q/k tiles 0-1 and v3i for kb 0-7, all of which
        # live in token columns 0-1023 (the h0 DMA halves). Tiles 2/3 stream
        # in via stage-(0,0) slots while attention groups 0-3 already run.
        qkv_passA(0, 0)
        qkv_passA(0, 1)
        sv0 = qkv_ln(0)
        qkv_ft(0, 0, sv0)
        qkv_ft(0, 1, sv0)
        for p in range(4):
            v_pair(p)

        # ---------- software-pipelined attention group stream ----------
        # Template B (steady state), per group g: pexp(g) [Act], S(g+1) [PE],
        # PV(g-1) [PE], slot-item(g). Deferring PV one group lets the
        # in-order PE queue run the Act-critical S matmuls immediately after
        # the s-bank frees, so the next pexp is never stuck behind PV or
        # filler work. Template A (warm-up stages whose slot items produce
        # operands of upcoming S/PV matmuls) instead runs slot(g) BEFORE
        # S(g+1)/PV(g), keeping producer items ahead of their consumers on
        # the in-order PE queue (emitting a consumer first would deadlock).
        sv_box = {}

        def mk(fn, *a):
            return lambda: fn(*a)

        def mk_ln(h, t0, t1):
            def run():
                sv_box[h] = qkv_ln(h)
                qkv_ft(h, t0, sv_box[h])
                if t1 is not None:
                    qkv_ft(h, t1, sv_box[h])
            return run

        def mk_ft(h, t):
            return lambda: qkv_ft(h, t, sv_box[h])

        # (h, qt, template, slots); epilogue of the previous stage is
        # auto-prepended (2 slots).
        def seq(*fns):
            def run():
                for f in fns:
                    f()
            return run

        STAGES = [
            # (0,0) is template A: its slots produce kT tiles 2/3 and v3i
            # kb 8-15, consumed by this very stage's S(g4+)/PV(g4+).
            (0, 0, "A", [mk(qkv_passA_mm, 0, 2), mk(qkv_passA_ve, 0, 2),
                         mk_ln(0, 2, None), mk(qkv_passA_mm, 0, 3),
                         seq(mk(v_pair, 4), mk(v_pair, 5)),
                         seq(mk(qkv_passA_ve, 0, 3), mk_ln(0, 3, None)),
                         mk(v_pair, 6), mk(v_pair, 7)]),
            (0, 1, "B", [mk(qkv_passA_mm, 1, 0), mk(qkv_passA_ve, 1, 0),
                         mk(qkv_passA_mm, 1, 1), mk(qkv_passA_ve, 1, 1)]),
            (0, 2, "B", [mk(qkv_passA_mm, 1, 2), mk(qkv_passA_ve, 1, 2),
                         mk(qkv_passA_mm, 1, 3), mk(qkv_passA_ve, 1, 3),
                         mk_ln(1, 0, 1), mk_ft(1, 2)]),
            (0, 3, "B", [mk_ft(1, 3)]),
            (1, 0, "B", [mk(qkv_passA_mm, 2, 0), mk(qkv_passA_ve, 2, 0),
                         mk(qkv_passA_mm, 2, 1), mk(qkv_passA_ve, 2, 1)]),
            (1, 1, "B", [mk(qkv_passA_mm, 2, 2), mk(qkv_passA_ve, 2, 2),
                         mk(qkv_passA_mm, 2, 3), mk(qkv_passA_ve, 2, 3)]),
            (1, 2, "B", [mk_ln(2, 0, 1), mk_ft(2, 2), mk_ft(2, 3)]),
            (1, 3, "B", []),
            (2, 0, "B", []),
            (2, 1, "B", [mk(proj_tt, 0), mk(proj_tt, 1), mk(proj_tt, 2),
                         mk(proj_tt, 3)]),
            (2, 2, "B", [mk(proj_tt, 4), mk(proj_tt, 5), mk(proj_tt, 6),
                         mk(proj_tt, 7)]),
            (2, 3, "B", [mk(proj_tt, 8), mk(proj_tt, 9), mk(proj_tt, 10),
                         mk(proj_tt, 11)]),
        ]
        NS = len(STAGES)

        def S_of(si, g):
            h, qt, _, _ = STAGES[si]
            s_ps = (sA if g % 2 == 0 else sB).tile([128, 1024], F32, tag="s")
            qs = slice(qt * 512, (qt + 1) * 512)
            for j in range(2):
                kb = 2 * g + j
                mm(s_ps[:, j * 512:(j + 1) * 512],
                   kT(h)[:, kb * 128:(kb + 1) * 128], qT(h)[:, qs],
                   True, True)
            return s_ps

        def mk_pv(h, o_t, px, g):
            def run():
                for j in range(2):
                    kb = 2 * g + j
                    for qc in range(4):
                        mm(o_t[:, qc * 65:qc * 65 + 65],
                           px[:, j * 512 + qc * 128:j * 512 + (qc + 1) * 128],
                           v3i[:, (h * KB + kb) * 65:(h * KB + kb + 1) * 65],
                           kb == 0, kb == KB - 1)
            return run

        prev = None        # (h, qt, o_t) of previous stage, epilogue pending
        pv_pending = None  # deferred PV of the previous group
        s_cur = S_of(0, 0)
        for si in range(NS):
            h, qt, tmpl, items = STAGES[si]
            slots = list(items)
            if prev is not None:
                ph, pqt, po_t = prev
                box = {}

                def mk_ea(ph=ph, pqt=pqt, po_t=po_t, box=box):
                    def run():
                        box["o_n"] = epi_a(ph, pqt, po_t)
                    return run

                def mk_eb(ph=ph, pqt=pqt, po_t=po_t, box=box):
                    return lambda: epi_b(ph, pqt, po_t, box["o_n"])

                slots = [mk_ea(), mk_eb()] + slots
            assert len(slots) <= 8, (si, len(slots))
            o_t = (oA if si % 2 == 0 else oB).tile([128, 512], F32, tag="o")
            for g in range(8):
                px = pe.tile([128, 1024], BF16, tag="pexp")
                nc.scalar.activation(px[:], s_cur[:], AF.Exp,
                                     bias=0.0, scale=0.125)
                if tmpl == "A":
                    if pv_pending is not None:
                        pv_pending()
                        pv_pending = None
                    if g < len(slots):
                        slots[g]()
                    if g < 7:
                        s_cur = S_of(si, g + 1)
                    elif si + 1 < NS:
                        s_cur = S_of(si + 1, 0)
                    mk_pv(h, o_t, px, g)()
                else:
                    if g < 7:
                        s_cur = S_of(si, g + 1)
                    elif si + 1 < NS:
                        s_cur = S_of(si + 1, 0)
                    if pv_pending is not None:
                        pv_pending()
                    pv_pending = mk_pv(h, o_t, px, g)
                    if g < len(slots):
                        slots[g]()
            prev = (h, qt, o_t)

        if pv_pending is not None:
            pv_pending()  # last PV group
        # tail: last epilogue + last projection q-tile
        ph, pqt, po_t = prev
        o_n = epi_a(ph, pqt, po_t)
        epi_b(ph, pqt, po_t, o_n)
        for tt in range(12, 16):
            proj_tt(tt)

    if split_waits:
        _split_waits(nc)
    return nc


def _split_waits(nc):
    """This walrus build lowers at most one sync-wait per instruction (the
    matmul LDW struct rejects 2+). Move excess waits onto NoOps inserted
    just before, on the same engine queue — queues are in-order, so the
    constraint is preserved exactly."""
    k = 0
    for fn in nc.m.functions:
        for bb in fn.blocks:
            il = bb.instructions
            idx = 0
            while idx < len(il):
                inst = il[idx]
                si = inst.sync_info
                eng = getattr(inst, "engine", None)
                if (si is not None and len(si.on_wait) > 1
                        and eng is not None
                        and str(eng) != "EngineType.Unassigned"):
                    waits = list(si.on_wait)
                    inst.sync_info = mybir.SyncInfo(
                        on_wait=[waits[-1]], on_update=list(si.on_update))
                    for w in waits[:-1]:
                        nop = mybir.InstNoOp(
                            name=f"I-waitnop-{k}", engine=eng, ins=[], outs=[],
                            sync_info=mybir.SyncInfo(on_wait=[w], on_update=[]))
                        k += 1
                        il.insert(idx, nop)
                        idx += 1
                idx += 1


def _bf16(a):
    return np.asarray(a, dtype=np.float32).astype(mybir.dt.np(BF16))


def _prep_core_inputs(core, x, rope_cos, rope_sin, qkv_kernel, qkv_bias,
                      proj_kernel, proj_bias, q_norm_w, k_norm_w):
    b = core // 4
    heads = [3 * (core % 4) + i for i in range(HP)]

    wq = qkv_kernel.reshape(C, 3, H, HD)
    bq = qkv_bias.reshape(3, H, HD)

    xT = np.ascontiguousarray(x[b].T, dtype=np.float32)

    wqk2 = np.empty((128, CCH * HP * 128), np.float32)
    wv2 = np.empty((128, CCH * HP * 64), np.float32)
    for c in range(CCH):
        rows = slice(c * 128, (c + 1) * 128)
        for i, h in enumerate(heads):
            base = c * HP * 128 + i * 128
            wqk2[:, base:base + 64] = wq[rows, 0, h][:, PERM]
            wqk2[:, base + 64:base + 128] = wq[rows, 1, h][:, PERM]
            wv2[:, c * HP * 64 + i * 64:c * HP * 64 + (i + 1) * 64] = \
                wq[rows, 2, h]

    bqkbv = np.empty((1, HP * 192), np.float32)
    for i, h in enumerate(heads):
        bqkbv[0, i * 128:i * 128 + 64] = bq[0, h, PERM]
        bqkbv[0, i * 128 + 64:(i + 1) * 128] = bq[1, h, PERM]
        bqkbv[0, HP * 128 + i * 64:HP * 128 + (i + 1) * 64] = bq[2, h]

    cosT = rope_cos.T  # (HD, N)
    sinT = rope_sin.T
    cos2w = np.empty((128, N), np.float32)
    sinSw = np.empty((128, N), np.float32)
    cos2w[0:64] = cosT[PERM] * q_norm_w[PERM][:, None]
    cos2w[64:128] = cosT[PERM] * k_norm_w[PERM][:, None]
    sinSw[0:64] = SIGN[:, None] * sinT[PERM] * q_norm_w[PERM][:, None]
    sinSw[64:128] = SIGN[:, None] * sinT[PERM] * k_norm_w[PERM][:, None]

    rows = np.concatenate([np.arange(h * HD, (h + 1) * HD) for h in heads])
    wp = np.ascontiguousarray(proj_kernel[rows, :], dtype=np.float32)

    identd = np.eye(128, dtype=np.float32)

    return {"xT": _bf16(xT), "wqk2": _bf16(wqk2), "wv2": _bf16(wv2),
            "bqkbv": _bf16(bqkbv),
            "cos2w": _bf16(cos2w), "sinSw": _bf16(sinSw),
            "wp": _bf16(wp), "identd": identd}


def kernel(x, rope_cos, rope_sin, qkv_kernel, qkv_bias, proj_kernel,
           proj_bias, q_norm_w, k_norm_w, _trace=False):
    args = [np.asarray(a, dtype=np.float32) for a in
            (x, rope_cos, rope_sin, qkv_kernel, qkv_bias, proj_kernel,
             proj_bias, q_norm_w, k_norm_w)]
    in_maps = [_prep_core_inputs(c, *args) for c in range(NCORES)]

    if "nc" not in _NC_CACHE:
        _NC_CACHE["nc"] = build_nc()
    nc = _NC_CACHE["nc"]

    res = run_bass_kernel_spmd(nc, in_maps, core_ids=list(range(NCORES)),
                               trace=_trace)
    parts = [np.asarray(res.results[c]["out"], dtype=np.float32)
             for c in range(NCORES)]
    out = np.empty((B, N, C), np.float32)
    pb = np.asarray(proj_bias, dtype=np.float32)
    for b in range(B):
        out[b] = parts[4 * b] + parts[4 * b + 1] + parts[4 * b + 2] + parts[4 * b + 3] + pb
    if _trace:
        kernel.last_results = res
    return out


# revision 21
# speedup vs baseline: 1.0067x; 1.0067x over previous
"""Multi-head attention (RMSNorm-QK + RoPE + softmax + proj) on 8 Trainium2 cores.

Sharding: core c handles batch b = c//4 and heads [3*(c%4), 3*(c%4)+3).
Each core computes qkv for its heads, flash-style attention, and a partial
projection over its heads' channels; the host sums the 4 partials per batch.

Design notes:
 - all matmul moving operands are bf16 (1 cyc/row on the PE at any free
   size), weights/x/tables DMA'd as bf16 to halve input traffic.
 - PV matmul in flipped [q,d] orientation (px stationary, v moving, 65-row
   outputs incl. a ones-column for the softmax denominator), halving PV cost
   vs the [d,q] orientation.
 - softmax epilogue: DVE reciprocal of the denominator column + per-q-chunk
   tensor_scalar, then a small PE transpose (through scratch space in the o
   PSUM bank) back to [d,q] for the projection.
 - q^T/k^T layout [head_dim, tokens]; head-dim rows permuted so the RoPE
   half-swap is an intra-quadrant stream_shuffle.
 - RMS-norm: sum(q^2) via ones-pair matmul; rsqrt = exp(-0.5*ln(x)); one ACT
   table set for the whole kernel.
 - the attention inner loop is a software-pipelined stream of 96 S->exp->PV
   groups; the S matmuls of group g+1 are emitted before the filler work of
   group g so the in-order PE queue always serves the Act-critical path
   first. qkv for heads 1,2, v-compute, epilogues and projection are diced
   into ~1-2us work items placed into one slot per group.
 - elementwise work is split between DVE (shuffle, squares, adds, epilogue)
   and the Pool/GPSIMD engine (cos-mul, k-scale, PSUM->SBUF copies).
"""
import sys

for _p in ("/opt/trn_rl_repo", "/opt/trn_rl_repo/concourse"):
    if _p not in sys.path:
        sys.path.insert(0, _p)

import numpy as np
from contextlib import ExitStack

import concourse.bass as bass
import concourse.tile as tile
import concourse.mybir as mybir
from concourse.bass_utils import run_bass_kernel_spmd

F32 = mybir.dt.float32
F32R = mybir.dt.float32r
BF16 = mybir.dt.bfloat16
AF = mybir.ActivationFunctionType

B, N, C = 2, 2048, 768
H, HD = 12, 64
HP = 3            # heads per core
NCORES = 8
CCH = C // 128    # 6 contraction chunks
NT = N // 512     # 4 token tiles of 512
KB = N // 128     # 16 k-blocks of 128
EPS = 1e-6

SWAP_MASK = [(i + 16) % 32 for i in range(32)]
# head-dim permutation: pair-exchange (d <-> d+32) becomes intra-quadrant
PERM = np.concatenate([np.arange(0, 16), np.arange(32, 48),
                       np.arange(16, 32), np.arange(48, 64)])
SIGN = np.where(PERM < 32, -1.0, 1.0).astype(np.float32)

_NC_CACHE = {}


def build_nc(split_waits=True):
    nc = bass.Bass(target_bir_lowering=True)
    xT = nc.declare_dram_parameter("xT", [C, N], BF16, isOutput=False)
    # weights packed chunk-horizontal so one DMA covers several chunks
    wqk2 = nc.declare_dram_parameter("wqk2", [128, CCH * HP * 128], BF16,
                                     isOutput=False)
    wv2 = nc.declare_dram_parameter("wv2", [128, CCH * HP * 64], BF16,
                                    isOutput=False)
    bqkbv = nc.declare_dram_parameter("bqkbv", [1, HP * 192], BF16,
                                      isOutput=False)
    cos2w = nc.declare_dram_parameter("cos2w", [128, N], BF16, isOutput=False)
    sinSw = nc.declare_dram_parameter("sinSw", [128, N], BF16, isOutput=False)
    wp = nc.declare_dram_parameter("wp", [HP * HD, C], BF16, isOutput=False)
    identd = nc.declare_dram_parameter("identd", [128, 128], F32R,
                                       isOutput=False)
    out = nc.declare_dram_parameter("out", [N, C], BF16, isOutput=True)

    with tile.TileContext(nc) as tc, ExitStack() as ctx:
        sb = ctx.enter_context(tc.tile_pool(name="sb", bufs=1))
        tp = ctx.enter_context(tc.tile_pool(name="tp", bufs=2))
        pe = ctx.enter_context(tc.tile_pool(name="pe", bufs=3))   # pexp
        tp1 = ctx.enter_context(tc.tile_pool(name="tp1", bufs=2))
        fps = ctx.enter_context(tc.tile_pool(name="fps", bufs=2, space="PSUM"))
        sA = ctx.enter_context(tc.tile_pool(name="sA", bufs=1, space="PSUM"))
        sB = ctx.enter_context(tc.tile_pool(name="sB", bufs=1, space="PSUM"))
        oA = ctx.enter_context(tc.tile_pool(name="oA", bufs=1, space="PSUM"))
        oB = ctx.enter_context(tc.tile_pool(name="oB", bufs=1, space="PSUM"))

        # ---------- prologue ----------
        # x half-chunks alternate the two HWDGE queues (SP/Act) so the qkv
        # matmuls chase the loads; big weight packs ride SWDGE (Pool);
        # small constants are memset-derived to keep the DMA count low
        # (each HWDGE issue serializes ~0.65us on the single HWDGE device).
        # memset-derived constants (emitted first: Pool runs these before
        # its SWDGE issue backlog so the RoPE chain isn't gated on them)
        ones_row = sb.tile([1, 512], BF16, tag="ones_row")
        nc.gpsimd.memset(ones_row[:], 1.0)
        onesp = sb.tile([128, 2], BF16, tag="onesp")
        nc.gpsimd.memset(onesp[:], 0.0)
        nc.gpsimd.memset(onesp[0:64, 0:1], 1.0)
        nc.gpsimd.memset(onesp[64:128, 1:2], 1.0)
        eps_t = sb.tile([128, 1], F32, tag="eps")
        nc.gpsimd.memset(eps_t[:], EPS)
        # v3i: per (head, kb) a [128, 65] block: v columns 0:64, ones col 64
        v3i = sb.tile([128, HP * KB * 65], BF16, tag="v3i")
        nc.gpsimd.memset(
            v3i[:].rearrange("p (b n) -> p b n", n=65)[:, :, 64:65], 1.0)
        s_sb = sb.tile([128, 512], F32, tag="s_sb")
        nc.gpsimd.memset(s_sb[:], 1.0)

        bqkbv_sb = sb.tile([1, HP * 192], BF16, tag="bqkbv")
        nc.gpsimd.dma_start(bqkbv_sb[:], bqkbv[:, :])
        wqk_sb = sb.tile([128, CCH * HP * 128], BF16, tag="wqk")
        nc.gpsimd.dma_start(wqk_sb[:, 0:HP * 384], wqk2[:, 0:HP * 384])
        nc.gpsimd.dma_start(wqk_sb[:, HP * 384:], wqk2[:, HP * 384:])
        cos_sb = sb.tile([128, N], BF16, tag="cos")
        nc.gpsimd.dma_start(cos_sb[:, 0:1024], cos2w[:, 0:1024])
        sin_sb = sb.tile([128, N], BF16, tag="sin")
        nc.gpsimd.dma_start(sin_sb[:, 0:1024], sinSw[:, 0:1024])
        wv_sb = sb.tile([128, CCH * HP * 64], BF16, tag="wv")
        nc.gpsimd.dma_start(wv_sb[:], wv2[:, :])
        nc.gpsimd.dma_start(cos_sb[:, 1024:2048], cos2w[:, 1024:2048])
        nc.gpsimd.dma_start(sin_sb[:, 1024:2048], sinSw[:, 1024:2048])
        xs = []
        for c in range(CCH):
            t = sb.tile([128, N], BF16, tag=f"x{c}")
            nc.sync.dma_start(t[:, 0:1024], xT[c * 128:(c + 1) * 128, 0:1024])
            nc.scalar.dma_start(t[:, 1024:2048],
                                xT[c * 128:(c + 1) * 128, 1024:2048])
            xs.append(t)
        wp0_sb = sb.tile([128, C], BF16, tag="wp0")
        nc.sync.dma_start(wp0_sb[:], wp[0:128, :])
        wp1_sb = sb.tile([64, C], BF16, tag="wp1")
        nc.sync.dma_start(wp1_sb[:], wp[128:192, :])
        ident = sb.tile([128, 128], F32R, tag="ident")
        nc.sync.dma_start(ident[:], identd[:, :])

        def wqk_c(c, h):
            return wqk_sb[:, c * HP * 128 + h * 128:c * HP * 128 + (h + 1) * 128]

        def wv_c(c):
            return wv_sb[:, c * HP * 64:(c + 1) * HP * 64]

        bqk_sb = bqkbv_sb[:, 0:HP * 128]
        bv_sb = bqkbv_sb[:, HP * 128:HP * 192]


        # qT/kT packed by head pairs so S-matmul operands share a base partition
        q12 = sb.tile([128, N], BF16, tag="q12")
        k12 = sb.tile([128, N], BF16, tag="k12")
        q3 = sb.tile([64, N], BF16, tag="q3")
        k3 = sb.tile([64, N], BF16, tag="k3")

        def qT(h):
            return (q12[0:64], q12[64:128], q3[:])[h]

        def kT(h):
            return (k12[0:64], k12[64:128], k3[:])[h]

        oall_a = sb.tile([128, N], BF16, tag="oall_a")   # heads 0,1 O^T
        oall_b = sb.tile([64, N], BF16, tag="oall_b")    # head 2 O^T
        t4_all = sb.tile([128, N], BF16, tag="t4_all")

        def mm(out_ap, lhsT, rhs, start, stop):
            nc.tensor.matmul(out_ap, lhsT, rhs,
                             start=start, stop=stop, skip_group_check=True)

        # ---------- qkv work items (split into matmul and vector halves) ----
        qk_box = {}

        def qkv_passA_mm(h, t):
            ts = slice(t * 512, (t + 1) * 512)
            qk_ps = fps.tile([128, 512], F32, tag="flex")
            for c in range(CCH):
                mm(qk_ps[:], wqk_c(c, h), xs[c][:, ts], c == 0, False)
            mm(qk_ps[:], bqk_sb[:, h * 128:(h + 1) * 128], ones_row[:],
               False, True)
            qk_box[(h, t)] = qk_ps

        def qkv_passA_ve(h, t):
            ts = slice(t * 512, (t + 1) * 512)
            qk_ps = qk_box.pop((h, t))
            t1 = tp1.tile([128, 512], BF16, tag="t1")
            nc.vector.tensor_mul(t1[:], qk_ps[:], cos_sb[:, ts])
            t2 = tp.tile([128, 512], BF16, tag="t2")
            nc.vector.stream_shuffle(t2[:], qk_ps[:], SWAP_MASK)
            sq = tp.tile([128, 512], BF16, tag="sq")
            nc.gpsimd.tensor_mul(sq[:], t2[:], t2[:])
            t3 = tp.tile([128, 512], BF16, tag="t3")
            nc.gpsimd.tensor_mul(t3[:], t2[:], sin_sb[:, ts])
            mm(qk_ps[0:2, :], onesp[:], sq[:], True, True)
            nc.vector.tensor_copy(s_sb[32 * t:32 * t + 2, :], qk_ps[0:2, :])
            nc.gpsimd.tensor_add(t4_all[:, ts], t1[:], t3[:])

        def qkv_passA(h, t):
            qkv_passA_mm(h, t)
            qkv_passA_ve(h, t)

        def qkv_ln(h):
            lnv = tp1.tile([128, 512], F32, tag="lnv")
            nc.scalar.activation(lnv[:], s_sb[:], AF.Ln,
                                 bias=eps_t[:], scale=1.0 / HD)
            sv = tp1.tile([128, 512], BF16, tag="sv")
            nc.scalar.activation(sv[:], lnv[:], AF.Exp, bias=0.0, scale=-0.5)
            return sv

        def qkv_ft(h, t, sv):
            # broadcast the per-token rsqrt rows to 64 partitions via DMA
            # (stride-0 source) so the q/k scale muls run as all-bf16 SBUF
            # DVE ops in 2x mode.
            ts = slice(t * 512, (t + 1) * 512)
            bc = tp.tile([128, 512], BF16, tag="bc")
            nc.sync.dma_start(
                bc[0:64, :],
                sv[32 * t:32 * t + 1, :][:, None, :]
                .broadcast_to((1, 64, 512)))
            nc.sync.dma_start(
                bc[64:128, :],
                sv[32 * t + 1:32 * t + 2, :][:, None, :]
                .broadcast_to((1, 64, 512)))
            nc.vector.tensor_mul(qT(h)[:, ts], t4_all[0:64, ts], bc[0:64, :])
            nc.vector.tensor_mul(kT(h)[:, ts], t4_all[64:128, ts],
                                 bc[64:128, :])

        # ---------- v for all heads, one tt-pair (mm and copy halves) -------
        v_box = {}

        def v_pair_mm(p):
            v_ps = fps.tile([128, 384], F32, tag="flex")
            for i, tt in enumerate((2 * p, 2 * p + 1)):
                for h in range(HP):
                    vs = slice((i * HP + h) * 64, (i * HP + h + 1) * 64)
                    for c in range(CCH):
                        mm(v_ps[:, vs], xs[c][:, tt * 128:(tt + 1) * 128],
                           wv_c(c)[:, h * 64:(h + 1) * 64], c == 0, False)
                    mm(v_ps[:, vs], ones_row[0:1, 0:128],
                       bv_sb[:, h * 64:(h + 1) * 64], False, True)
            v_box[p] = v_ps

        def v_pair_cp(p):
            v_ps = v_box.pop(p)
            dst = v3i[:].rearrange("p (g k n) -> p g k n", g=HP, k=KB)
            nc.vector.tensor_copy(
                dst[:, :, 2 * p:2 * p + 2, 0:64],
                v_ps[:].rearrange("p (i g n) -> p g i n", i=2, g=HP))

        def v_pair(p):
            v_pair_mm(p)
            v_pair_cp(p)

        # ---------- epilogue + projection work items ----------
        def epi_a(h, qt, o_t):
            rec = tp1.tile([128, 4], F32, tag="rec")
            nc.vector.reciprocal_approx_fast(rec[:], o_t[:, 64:64 + 4 * 65:65])
            o_n = tp.tile([128, 256], F32R, tag="o_n")
            for qc in range(4):
                nc.vector.tensor_scalar_mul(
                    o_n[:, qc * 64:(qc + 1) * 64],
                    o_t[:, qc * 65:qc * 65 + 64], rec[:, qc:qc + 1])
            return o_n

        def epi_b(h, qt, o_t, o_n, tail=False):
            for qc in range(4):
                # transpose [128q, 64d] -> [64d, 128q] via PE, scratch in
                # the unused tail of the o PSUM bank
                nc.tensor.matmul(o_t[0:64, 384:512].bitcast(F32R),
                                 o_n[:, qc * 64:(qc + 1) * 64],
                                 ident[:],
                                 start=True, stop=True, is_transpose=True,
                                 skip_group_check=True)
                cs = slice(qt * 512 + qc * 128, qt * 512 + (qc + 1) * 128)
                dst = oall_a[h * 64:(h + 1) * 64, cs] if h < 2 \
                    else oall_b[:, cs]
                if tail and qc % 2 == 1:
                    nc.scalar.copy(dst, o_t[0:64, 384:512])
                else:
                    nc.vector.tensor_copy(dst, o_t[0:64, 384:512])

        def proj_tt(tt, tail=False):
            po = tp.tile([128, C], BF16, tag="po")
            for half in range(2):
                cs = slice(half * 384, (half + 1) * 384)
                p_ps = fps.tile([128, 384], F32, tag="flex")
                mm(p_ps[:], oall_a[:, tt * 128:(tt + 1) * 128],
                   wp0_sb[:, cs], True, False)
                mm(p_ps[:], oall_b[:, tt * 128:(tt + 1) * 128],
                   wp1_sb[:, cs], False, True)
                if tail and half == 1:
                    nc.scalar.copy(po[:, cs], p_ps[:])
                else:
                    nc.vector.tensor_copy(po[:, cs], p_ps[:])
            nc.sync.dma_start(out[tt * 128:(tt + 1) * 128, :], po[:])

        # ---------- lead-in: tiles 0/1 of head 0 (x h0-halves only) ----------
        # First pexp only needs q/k tiles 0-1 and v3i for kb 0-7, all of which
        # live in token columns 0-1023 (the h0 DMA halves). Tiles 2/3 stream
        # in via stage-(0,0) slots while attention groups 0-3 already run.
        qkv_passA(0, 0)
        qkv_passA(0, 1)
        sv0 = qkv_ln(0)
        qkv_ft(0, 0, sv0)
        qkv_ft(0, 1, sv0)
        for p in range(4):
            v_pair(p)

        # ---------- software-pipelined attention group stream ----------
        # Template B (steady state), per group g: pexp(g) [Act], S(g+1) [PE],
        # PV(g-1) [PE], slot-item(g). Deferring PV one group lets the
        # in-order PE queue run the Act-critical S matmuls immediately after
        # the s-bank frees, so the next pexp is never stuck behind PV or
        # filler work. Template A (warm-up stages whose slot items produce
        # operands of upcoming S/PV matmuls) instead runs slot(g) BEFORE
        # S(g+1)/PV(g), keeping producer items ahead of their consumers on
        # the in-order PE queue (emitting a consumer first would deadlock).
        sv_box = {}

        def mk(fn, *a):
            return lambda: fn(*a)

        def mk_ln(h, t0, t1):
            def run():
                sv_box[h] = qkv_ln(h)
                qkv_ft(h, t0, sv_box[h])
                if t1 is not None:
                    qkv_ft(h, t1, sv_box[h])
            return run

        def mk_ft(h, t):
            return lambda: qkv_ft(h, t, sv_box[h])

        # (h, qt, template, slots); epilogue of the previous stage is
        # auto-prepended (2 slots).
        def seq(*fns):
            def run():
                for f in fns:
                    f()
            return run

        STAGES = [
            # (0,0) is template A: its slots produce kT tiles 2/3 and v3i
            # kb 8-15, consumed by this very stage's S(g4+)/PV(g4+).
            (0, 0, "A", [mk(qkv_passA_mm, 0, 2), mk(qkv_passA_ve, 0, 2),
                         mk_ln(0, 2, None), mk(qkv_passA_mm, 0, 3),
                         seq(mk(v_pair, 4), mk(v_pair, 5)),
                         seq(mk(qkv_passA_ve, 0, 3), mk_ln(0, 3, None)),
                         mk(v_pair, 6), mk(v_pair, 7)]),
            (0, 1, "B", [mk(qkv_passA_mm, 1, 0), mk(qkv_passA_ve, 1, 0),
                         mk(qkv_passA_mm, 1, 1), mk(qkv_passA_ve, 1, 1)]),
            (0, 2, "B", [mk(qkv_passA_mm, 1, 2), mk(qkv_passA_ve, 1, 2),
                         mk(qkv_passA_mm, 1, 3), mk(qkv_passA_ve, 1, 3),
                         mk_ln(1, 0, 1), mk_ft(1, 2)]),
            (0, 3, "B", [mk_ft(1, 3)]),
            (1, 0, "B", [mk(qkv_passA_mm, 2, 0), mk(qkv_passA_ve, 2, 0),
                         mk(qkv_passA_mm, 2, 1), mk(qkv_passA_ve, 2, 1)]),
            (1, 1, "B", [mk(qkv_passA_mm, 2, 2), mk(qkv_passA_ve, 2, 2),
                         mk(qkv_passA_mm, 2, 3), mk(qkv_passA_ve, 2, 3)]),
            (1, 2, "B", [mk_ln(2, 0, 1), mk_ft(2, 2), mk_ft(2, 3)]),
            (1, 3, "B", []),
            (2, 0, "B", []),
            (2, 1, "B", [mk(proj_tt, 0), mk(proj_tt, 1), mk(proj_tt, 2),
                         mk(proj_tt, 3)]),
            (2, 2, "B", [mk(proj_tt, 4), mk(proj_tt, 5), mk(proj_tt, 6),
                         mk(proj_tt, 7)]),
            (2, 3, "B", [mk(proj_tt, 8), mk(proj_tt, 9), mk(proj_tt, 10),
                         mk(proj_tt, 11)]),
        ]
        NS = len(STAGES)

        def S_of(si, g):
            h, qt, _, _ = STAGES[si]
            s_ps = (sA if g % 2 == 0 else sB).tile([128, 1024], F32, tag="s")
            qs = slice(qt * 512, (qt + 1) * 512)
            for j in range(2):
                kb = 2 * g + j
                mm(s_ps[:, j * 512:(j + 1) * 512],
                   kT(h)[:, kb * 128:(kb + 1) * 128], qT(h)[:, qs],
                   True, True)
            return s_ps

        def mk_pv(h, o_t, px, g):
            def run():
                for j in range(2):
                    kb = 2 * g + j
                    for qc in range(4):
                        mm(o_t[:, qc * 65:qc * 65 + 65],
                           px[:, j * 512 + qc * 128:j * 512 + (qc + 1) * 128],
                           v3i[:, (h * KB + kb) * 65:(h * KB + kb + 1) * 65],
                           kb == 0, kb == KB - 1)
            return run

        prev = None        # (h, qt, o_t) of previous stage, epilogue pending
        pv_pending = None  # deferred PV of the previous group
        s_cur = S_of(0, 0)
        for si in range(NS):
            h, qt, tmpl, items = STAGES[si]
            slots = list(items)
            if prev is not None:
                ph, pqt, po_t = prev
                box = {}

                def mk_ea(ph=ph, pqt=pqt, po_t=po_t, box=box):
                    def run():
                        box["o_n"] = epi_a(ph, pqt, po_t)
                    return run

                def mk_eb(ph=ph, pqt=pqt, po_t=po_t, box=box):
                    return lambda: epi_b(ph, pqt, po_t, box["o_n"])

                slots = [mk_ea(), mk_eb()] + slots
            assert len(slots) <= 8, (si, len(slots))
            o_t = (oA if si % 2 == 0 else oB).tile([128, 512], F32, tag="o")
            for g in range(8):
                px = pe.tile([128, 1024], BF16, tag="pexp")
                nc.scalar.activation(px[:], s_cur[:], AF.Exp,
                                     bias=0.0, scale=0.125)
                if tmpl == "A":
                    if pv_pending is not None:
                        pv_pending()
                        pv_pending = None
                    if g < len(slots):
                        slots[g]()
                    if g < 7:
                        s_cur = S_of(si, g + 1)
                    elif si + 1 < NS:
                        s_cur = S_of(si + 1, 0)
                    mk_pv(h, o_t, px, g)()
                else:
                    if g < 7:
                        s_cur = S_of(si, g + 1)
                    elif si + 1 < NS:
                        s_cur = S_of(si + 1, 0)
                    if pv_pending is not None:
                        pv_pending()
                    pv_pending = mk_pv(h, o_t, px, g)
                    if g < len(slots):
                        slots[g]()
            prev = (h, qt, o_t)

        if pv_pending is not None:
            pv_pending()  # last PV group
        # tail: last epilogue + last projection q-tile
        ph, pqt, po_t = prev
        o_n = epi_a(ph, pqt, po_t)
        epi_b(ph, pqt, po_t, o_n, tail=True)
        for tt in range(12, 16):
            proj_tt(tt, tail=True)

    if split_waits:
        _split_waits(nc)
    return nc


def _split_waits(nc):
    """This walrus build lowers at most one sync-wait per instruction (the
    matmul LDW struct rejects 2+). Move excess waits onto NoOps inserted
    just before, on the same engine queue — queues are in-order, so the
    constraint is preserved exactly."""
    k = 0
    for fn in nc.m.functions:
        for bb in fn.blocks:
            il = bb.instructions
            idx = 0
            while idx < len(il):
                inst = il[idx]
                si = inst.sync_info
                eng = getattr(inst, "engine", None)
                if (si is not None and len(si.on_wait) > 1
                        and eng is not None
                        and str(eng) != "EngineType.Unassigned"):
                    waits = list(si.on_wait)
                    inst.sync_info = mybir.SyncInfo(
                        on_wait=[waits[-1]], on_update=list(si.on_update))
                    for w in waits[:-1]:
                        nop = mybir.InstNoOp(
                            name=f"I-waitnop-{k}", engine=eng, ins=[], outs=[],
                            sync_info=mybir.SyncInfo(on_wait=[w], on_update=[]))
                        k += 1
                        il.insert(idx, nop)
                        idx += 1
                idx += 1


def _bf16(a):
    return np.asarray(a, dtype=np.float32).astype(mybir.dt.np(BF16))


def _prep_core_inputs(core, x, rope_cos, rope_sin, qkv_kernel, qkv_bias,
                      proj_kernel, proj_bias, q_norm_w, k_norm_w):
    b = core // 4
    heads = [3 * (core % 4) + i for i in range(HP)]

    wq = qkv_kernel.reshape(C, 3, H, HD)
    bq = qkv_bias.reshape(3, H, HD)

    xT = np.ascontiguousarray(x[b].T, dtype=np.float32)

    wqk2 = np.empty((128, CCH * HP * 128), np.float32)
    wv2 = np.empty((128, CCH * HP * 64), np.float32)
    for c in range(CCH):
        rows = slice(c * 128, (c + 1) * 128)
        for i, h in enumerate(heads):
            base = c * HP * 128 + i * 128
            wqk2[:, base:base + 64] = wq[rows, 0, h][:, PERM]
            wqk2[:, base + 64:base + 128] = wq[rows, 1, h][:, PERM]
            wv2[:, c * HP * 64 + i * 64:c * HP * 64 + (i + 1) * 64] = \
                wq[rows, 2, h]

    bqkbv = np.empty((1, HP * 192), np.float32)
    for i, h in enumerate(heads):
        bqkbv[0, i * 128:i * 128 + 64] = bq[0, h, PERM]
        bqkbv[0, i * 128 + 64:(i + 1) * 128] = bq[1, h, PERM]
        bqkbv[0, HP * 128 + i * 64:HP * 128 + (i + 1) * 64] = bq[2, h]

    cosT = rope_cos.T  # (HD, N)
    sinT = rope_sin.T
    cos2w = np.empty((128, N), np.float32)
    sinSw = np.empty((128, N), np.float32)
    cos2w[0:64] = cosT[PERM] * q_norm_w[PERM][:, None]
    cos2w[64:128] = cosT[PERM] * k_norm_w[PERM][:, None]
    sinSw[0:64] = SIGN[:, None] * sinT[PERM] * q_norm_w[PERM][:, None]
    sinSw[64:128] = SIGN[:, None] * sinT[PERM] * k_norm_w[PERM][:, None]

    rows = np.concatenate([np.arange(h * HD, (h + 1) * HD) for h in heads])
    wp = np.ascontiguousarray(proj_kernel[rows, :], dtype=np.float32)

    identd = np.eye(128, dtype=np.float32)

    return {"xT": _bf16(xT), "wqk2": _bf16(wqk2), "wv2": _bf16(wv2),
            "bqkbv": _bf16(bqkbv),
            "cos2w": _bf16(cos2w), "sinSw": _bf16(sinSw),
            "wp": _bf16(wp), "identd": identd}


def kernel(x, rope_cos, rope_sin, qkv_kernel, qkv_bias, proj_kernel,
           proj_bias, q_norm_w, k_norm_w, _trace=False):
    args = [np.asarray(a, dtype=np.float32) for a in
            (x, rope_cos, rope_sin, qkv_kernel, qkv_bias, proj_kernel,
             proj_bias, q_norm_w, k_norm_w)]
    in_maps = [_prep_core_inputs(c, *args) for c in range(NCORES)]

    if "nc" not in _NC_CACHE:
        _NC_CACHE["nc"] = build_nc()
    nc = _NC_CACHE["nc"]

    res = run_bass_kernel_spmd(nc, in_maps, core_ids=list(range(NCORES)),
                               trace=_trace)
    parts = [np.asarray(res.results[c]["out"], dtype=np.float32)
             for c in range(NCORES)]
    out = np.empty((B, N, C), np.float32)
    pb = np.asarray(proj_bias, dtype=np.float32)
    for b in range(B):
        out[b] = parts[4 * b] + parts[4 * b + 1] + parts[4 * b + 2] + parts[4 * b + 3] + pb
    if _trace:
        kernel.last_results = res
    return out


# revision 23
# speedup vs baseline: 1.0232x; 1.0163x over previous
"""Multi-head attention (RMSNorm-QK + RoPE + softmax + proj) on 8 Trainium2 cores.

Sharding: core c handles batch b = c//4 and heads [3*(c%4), 3*(c%4)+3).
Each core computes qkv for its heads, flash-style attention, and a partial
projection over its heads' channels; the host sums the 4 partials per batch.

Design notes:
 - all matmul moving operands are bf16 (1 cyc/row on the PE at any free
   size), weights/x/tables DMA'd as bf16 to halve input traffic.
 - PV matmul in flipped [q,d] orientation (px stationary, v moving, 65-row
   outputs incl. a ones-column for the softmax denominator), halving PV cost
   vs the [d,q] orientation.
 - softmax epilogue: DVE reciprocal of the denominator column + per-q-chunk
   tensor_scalar, then a small PE transpose (through scratch space in the o
   PSUM bank) back to [d,q] for the projection.
 - q^T/k^T layout [head_dim, tokens]; head-dim rows permuted so the RoPE
   half-swap is an intra-quadrant stream_shuffle.
 - RMS-norm: sum(q^2) via ones-pair matmul; rsqrt = exp(-0.5*ln(x)); one ACT
   table set for the whole kernel.
 - the attention inner loop is a software-pipelined stream of 96 S->exp->PV
   groups; the S matmuls of group g+1 are emitted before the filler work of
   group g so the in-order PE queue always serves the Act-critical path
   first. qkv for heads 1,2, v-compute, epilogues and projection are diced
   into ~1-2us work items placed into one slot per group.
 - elementwise work is split between DVE (shuffle, squares, adds, epilogue)
   and the Pool/GPSIMD engine (cos-mul, k-scale, PSUM->SBUF copies).
"""
import sys

for _p in ("/opt/trn_rl_repo", "/opt/trn_rl_repo/concourse"):
    if _p not in sys.path:
        sys.path.insert(0, _p)

import numpy as np
from contextlib import ExitStack

import concourse.bass as bass
import concourse.tile as tile
import concourse.mybir as mybir
from concourse.bass_utils import run_bass_kernel_spmd

F32 = mybir.dt.float32
F32R = mybir.dt.float32r
BF16 = mybir.dt.bfloat16
AF = mybir.ActivationFunctionType

B, N, C = 2, 2048, 768
H, HD = 12, 64
HP = 3            # heads per core
NCORES = 8
CCH = C // 128    # 6 contraction chunks
NT = N // 512     # 4 token tiles of 512
KB = N // 128     # 16 k-blocks of 128
EPS = 1e-6

SWAP_MASK = [(i + 16) % 32 for i in range(32)]
# head-dim permutation: pair-exchange (d <-> d+32) becomes intra-quadrant
PERM = np.concatenate([np.arange(0, 16), np.arange(32, 48),
                       np.arange(16, 32), np.arange(48, 64)])
SIGN = np.where(PERM < 32, -1.0, 1.0).astype(np.float32)

_NC_CACHE = {}


def build_nc(split_waits=True):
    nc = bass.Bass(target_bir_lowering=True)
    xT = nc.declare_dram_parameter("xT", [C, N], BF16, isOutput=False)
    # weights packed chunk-horizontal so one DMA covers several chunks
    wqk2 = nc.declare_dram_parameter("wqk2", [128, CCH * HP * 128], BF16,
                                     isOutput=False)
    wv2 = nc.declare_dram_parameter("wv2", [128, CCH * HP * 64], BF16,
                                    isOutput=False)
    bqkbv = nc.declare_dram_parameter("bqkbv", [1, HP * 192], BF16,
                                      isOutput=False)
    cos2w = nc.declare_dram_parameter("cos2w", [128, N], BF16, isOutput=False)
    sinSw = nc.declare_dram_parameter("sinSw", [128, N], BF16, isOutput=False)
    wp = nc.declare_dram_parameter("wp", [HP * HD, C], BF16, isOutput=False)
    identd = nc.declare_dram_parameter("identd", [128, 128], F32R,
                                       isOutput=False)
    out = nc.declare_dram_parameter("out", [N, C], BF16, isOutput=True)

    with tile.TileContext(nc) as tc, ExitStack() as ctx:
        sb = ctx.enter_context(tc.tile_pool(name="sb", bufs=1))
        tp = ctx.enter_context(tc.tile_pool(name="tp", bufs=2))
        pe = ctx.enter_context(tc.tile_pool(name="pe", bufs=3))   # pexp
        tp1 = ctx.enter_context(tc.tile_pool(name="tp1", bufs=2))
        fps = ctx.enter_context(tc.tile_pool(name="fps", bufs=2, space="PSUM"))
        sA = ctx.enter_context(tc.tile_pool(name="sA", bufs=1, space="PSUM"))
        sB = ctx.enter_context(tc.tile_pool(name="sB", bufs=1, space="PSUM"))
        oA = ctx.enter_context(tc.tile_pool(name="oA", bufs=1, space="PSUM"))
        oB = ctx.enter_context(tc.tile_pool(name="oB", bufs=1, space="PSUM"))

        # ---------- prologue ----------
        # x half-chunks alternate the two HWDGE queues (SP/Act) so the qkv
        # matmuls chase the loads; big weight packs ride SWDGE (Pool);
        # small constants are memset-derived to keep the DMA count low
        # (each HWDGE issue serializes ~0.65us on the single HWDGE device).
        # memset-derived constants (emitted first: Pool runs these before
        # its SWDGE issue backlog so the RoPE chain isn't gated on them)
        ones_row = sb.tile([1, 512], BF16, tag="ones_row")
        nc.gpsimd.memset(ones_row[:], 1.0)
        onesp = sb.tile([128, 2], BF16, tag="onesp")
        nc.gpsimd.memset(onesp[:], 0.0)
        nc.gpsimd.memset(onesp[0:64, 0:1], 1.0)
        nc.gpsimd.memset(onesp[64:128, 1:2], 1.0)
        eps_t = sb.tile([128, 1], F32, tag="eps")
        nc.gpsimd.memset(eps_t[:], EPS)
        # v3i: per (head, kb) a [128, 65] block: v columns 0:64, ones col 64
        v3i = sb.tile([128, HP * KB * 65], BF16, tag="v3i")
        nc.gpsimd.memset(
            v3i[:].rearrange("p (b n) -> p b n", n=65)[:, :, 64:65], 1.0)
        s_sb = sb.tile([128, 512], F32, tag="s_sb")
        nc.gpsimd.memset(s_sb[:], 1.0)

        bqkbv_sb = sb.tile([1, HP * 192], BF16, tag="bqkbv")
        nc.gpsimd.dma_start(bqkbv_sb[:], bqkbv[:, :])
        wqk_sb = sb.tile([128, CCH * HP * 128], BF16, tag="wqk")
        nc.gpsimd.dma_start(wqk_sb[:, 0:HP * 384], wqk2[:, 0:HP * 384])
        nc.gpsimd.dma_start(wqk_sb[:, HP * 384:], wqk2[:, HP * 384:])
        cos_sb = sb.tile([128, N], BF16, tag="cos")
        nc.gpsimd.dma_start(cos_sb[:, 0:1024], cos2w[:, 0:1024])
        sin_sb = sb.tile([128, N], BF16, tag="sin")
        nc.gpsimd.dma_start(sin_sb[:, 0:1024], sinSw[:, 0:1024])
        wv_sb = sb.tile([128, CCH * HP * 64], BF16, tag="wv")
        nc.gpsimd.dma_start(wv_sb[:], wv2[:, :])
        nc.gpsimd.dma_start(cos_sb[:, 1024:2048], cos2w[:, 1024:2048])
        nc.gpsimd.dma_start(sin_sb[:, 1024:2048], sinSw[:, 1024:2048])
        # all h0 (token 0-1023) halves first — they gate the whole lead-in;
        # h1 halves are only needed by tiles 2/3 inside stage (0,0) slots
        xs = [sb.tile([128, N], BF16, tag=f"x{c}", name=f"x{c}")
              for c in range(CCH)]
        for c in range(CCH):
            q = nc.sync if c % 2 == 0 else nc.scalar
            q.dma_start(xs[c][:, 0:1024], xT[c * 128:(c + 1) * 128, 0:1024])
        for c in range(CCH):
            q = nc.scalar if c % 2 == 0 else nc.sync
            q.dma_start(xs[c][:, 1024:2048],
                        xT[c * 128:(c + 1) * 128, 1024:2048])
        wp0_sb = sb.tile([128, C], BF16, tag="wp0")
        nc.sync.dma_start(wp0_sb[:], wp[0:128, :])
        wp1_sb = sb.tile([64, C], BF16, tag="wp1")
        nc.sync.dma_start(wp1_sb[:], wp[128:192, :])
        ident = sb.tile([128, 128], F32R, tag="ident")
        nc.sync.dma_start(ident[:], identd[:, :])

        def wqk_c(c, h):
            return wqk_sb[:, c * HP * 128 + h * 128:c * HP * 128 + (h + 1) * 128]

        def wv_c(c):
            return wv_sb[:, c * HP * 64:(c + 1) * HP * 64]

        bqk_sb = bqkbv_sb[:, 0:HP * 128]
        bv_sb = bqkbv_sb[:, HP * 128:HP * 192]


        # qT/kT packed by head pairs so S-matmul operands share a base partition
        q12 = sb.tile([128, N], BF16, tag="q12")
        k12 = sb.tile([128, N], BF16, tag="k12")
        q3 = sb.tile([64, N], BF16, tag="q3")
        k3 = sb.tile([64, N], BF16, tag="k3")

        def qT(h):
            return (q12[0:64], q12[64:128], q3[:])[h]

        def kT(h):
            return (k12[0:64], k12[64:128], k3[:])[h]

        oall_a = sb.tile([128, N], BF16, tag="oall_a")   # heads 0,1 O^T
        oall_b = sb.tile([64, N], BF16, tag="oall_b")    # head 2 O^T
        t4_all = sb.tile([128, N], BF16, tag="t4_all")

        def mm(out_ap, lhsT, rhs, start, stop):
            nc.tensor.matmul(out_ap, lhsT, rhs,
                             start=start, stop=stop, skip_group_check=True)

        # ---------- qkv work items (split into matmul and vector halves) ----
        qk_box = {}

        def qkv_passA_mm(h, t):
            ts = slice(t * 512, (t + 1) * 512)
            qk_ps = fps.tile([128, 512], F32, tag="flex")
            for c in range(CCH):
                mm(qk_ps[:], wqk_c(c, h), xs[c][:, ts], c == 0, False)
            mm(qk_ps[:], bqk_sb[:, h * 128:(h + 1) * 128], ones_row[:],
               False, True)
            qk_box[(h, t)] = qk_ps

        def qkv_passA_ve(h, t):
            ts = slice(t * 512, (t + 1) * 512)
            qk_ps = qk_box.pop((h, t))
            t1 = tp1.tile([128, 512], BF16, tag="t1")
            nc.vector.tensor_mul(t1[:], qk_ps[:], cos_sb[:, ts])
            t2 = tp.tile([128, 512], BF16, tag="t2")
            nc.vector.stream_shuffle(t2[:], qk_ps[:], SWAP_MASK)
            sq = tp.tile([128, 512], BF16, tag="sq")
            nc.gpsimd.tensor_mul(sq[:], t2[:], t2[:])
            t3 = tp.tile([128, 512], BF16, tag="t3")
            nc.gpsimd.tensor_mul(t3[:], t2[:], sin_sb[:, ts])
            mm(qk_ps[0:2, :], onesp[:], sq[:], True, True)
            nc.vector.tensor_copy(s_sb[32 * t:32 * t + 2, :], qk_ps[0:2, :])
            nc.gpsimd.tensor_add(t4_all[:, ts], t1[:], t3[:])

        def qkv_passA(h, t):
            qkv_passA_mm(h, t)
            qkv_passA_ve(h, t)

        def qkv_ln(h):
            lnv = tp1.tile([128, 512], F32, tag="lnv")
            nc.scalar.activation(lnv[:], s_sb[:], AF.Ln,
                                 bias=eps_t[:], scale=1.0 / HD)
            sv = tp1.tile([128, 512], BF16, tag="sv")
            nc.scalar.activation(sv[:], lnv[:], AF.Exp, bias=0.0, scale=-0.5)
            return sv

        def qkv_ft(h, t, sv):
            # broadcast the per-token rsqrt rows to 64 partitions via DMA
            # (stride-0 source) so the q/k scale muls run as all-bf16 SBUF
            # DVE ops in 2x mode.
            ts = slice(t * 512, (t + 1) * 512)
            bc = tp.tile([128, 512], BF16, tag="bc")
            nc.sync.dma_start(
                bc[0:64, :],
                sv[32 * t:32 * t + 1, :][:, None, :]
                .broadcast_to((1, 64, 512)))
            nc.sync.dma_start(
                bc[64:128, :],
                sv[32 * t + 1:32 * t + 2, :][:, None, :]
                .broadcast_to((1, 64, 512)))
            nc.vector.tensor_mul(qT(h)[:, ts], t4_all[0:64, ts], bc[0:64, :])
            nc.vector.tensor_mul(kT(h)[:, ts], t4_all[64:128, ts],
                                 bc[64:128, :])

        # ---------- v for all heads, one tt-pair (mm and copy halves) -------
        v_box = {}

        def v_pair_mm(p):
            v_ps = fps.tile([128, 384], F32, tag="flex")
            for i, tt in enumerate((2 * p, 2 * p + 1)):
                for h in range(HP):
                    vs = slice((i * HP + h) * 64, (i * HP + h + 1) * 64)
                    for c in range(CCH):
                        mm(v_ps[:, vs], xs[c][:, tt * 128:(tt + 1) * 128],
                           wv_c(c)[:, h * 64:(h + 1) * 64], c == 0, False)
                    mm(v_ps[:, vs], ones_row[0:1, 0:128],
                       bv_sb[:, h * 64:(h + 1) * 64], False, True)
            v_box[p] = v_ps

        def v_pair_cp(p):
            v_ps = v_box.pop(p)
            dst = v3i[:].rearrange("p (g k n) -> p g k n", g=HP, k=KB)
            nc.vector.tensor_copy(
                dst[:, :, 2 * p:2 * p + 2, 0:64],
                v_ps[:].rearrange("p (i g n) -> p g i n", i=2, g=HP))

        def v_pair(p):
            v_pair_mm(p)
            v_pair_cp(p)

        # ---------- epilogue + projection work items ----------
        def epi_a(h, qt, o_t):
            rec = tp1.tile([128, 4], F32, tag="rec")
            nc.vector.reciprocal_approx_fast(rec[:], o_t[:, 64:64 + 4 * 65:65])
            o_n = tp.tile([128, 256], F32R, tag="o_n")
            for qc in range(4):
                nc.vector.tensor_scalar_mul(
                    o_n[:, qc * 64:(qc + 1) * 64],
                    o_t[:, qc * 65:qc * 65 + 64], rec[:, qc:qc + 1])
            return o_n

        def epi_b(h, qt, o_t, o_n, tail=False):
            for qc in range(4):
                # transpose [128q, 64d] -> [64d, 128q] via PE, scratch in
                # the unused tail of the o PSUM bank
                nc.tensor.matmul(o_t[0:64, 384:512].bitcast(F32R),
                                 o_n[:, qc * 64:(qc + 1) * 64],
                                 ident[:],
                                 start=True, stop=True, is_transpose=True,
                                 skip_group_check=True)
                cs = slice(qt * 512 + qc * 128, qt * 512 + (qc + 1) * 128)
                dst = oall_a[h * 64:(h + 1) * 64, cs] if h < 2 \
                    else oall_b[:, cs]
                if tail and qc % 2 == 1:
                    nc.scalar.copy(dst, o_t[0:64, 384:512])
                else:
                    nc.vector.tensor_copy(dst, o_t[0:64, 384:512])

        def proj_tt(tt, tail=False):
            po = tp.tile([128, C], BF16, tag="po")
            for half in range(2):
                cs = slice(half * 384, (half + 1) * 384)
                p_ps = fps.tile([128, 384], F32, tag="flex")
                mm(p_ps[:], oall_a[:, tt * 128:(tt + 1) * 128],
                   wp0_sb[:, cs], True, False)
                mm(p_ps[:], oall_b[:, tt * 128:(tt + 1) * 128],
                   wp1_sb[:, cs], False, True)
                if tail and half == 1:
                    nc.scalar.copy(po[:, cs], p_ps[:])
                else:
                    nc.vector.tensor_copy(po[:, cs], p_ps[:])
            nc.sync.dma_start(out[tt * 128:(tt + 1) * 128, :], po[:])

        # ---------- lead-in: tiles 0/1 of head 0 (x h0-halves only) ----------
        # First pexp only needs q/k tiles 0-1 and v3i for kb 0-7, all of which
        # live in token columns 0-1023 (the h0 DMA halves). Tiles 2/3 stream
        # in via stage-(0,0) slots while attention groups 0-3 already run.
        qkv_passA(0, 0)
        qkv_passA(0, 1)
        sv0 = qkv_ln(0)
        qkv_ft(0, 0, sv0)
        qkv_ft(0, 1, sv0)
        for p in range(4):
            v_pair(p)

        # ---------- software-pipelined attention group stream ----------
        # Template B (steady state), per group g: pexp(g) [Act], S(g+1) [PE],
        # PV(g-1) [PE], slot-item(g). Deferring PV one group lets the
        # in-order PE queue run the Act-critical S matmuls immediately after
        # the s-bank frees, so the next pexp is never stuck behind PV or
        # filler work. Template A (warm-up stages whose slot items produce
        # operands of upcoming S/PV matmuls) instead runs slot(g) BEFORE
        # S(g+1)/PV(g), keeping producer items ahead of their consumers on
        # the in-order PE queue (emitting a consumer first would deadlock).
        sv_box = {}

        def mk(fn, *a):
            return lambda: fn(*a)

        def mk_ln(h, t0, t1):
            def run():
                sv_box[h] = qkv_ln(h)
                qkv_ft(h, t0, sv_box[h])
                if t1 is not None:
                    qkv_ft(h, t1, sv_box[h])
            return run

        def mk_ft(h, t):
            return lambda: qkv_ft(h, t, sv_box[h])

        # (h, qt, template, slots); epilogue of the previous stage is
        # auto-prepended (2 slots).
        def seq(*fns):
            def run():
                for f in fns:
                    f()
            return run

        STAGES = [
            # (0,0) is template A: its slots produce kT tiles 2/3 and v3i
            # kb 8-15, consumed by this very stage's S(g4+)/PV(g4+).
            (0, 0, "A", [mk(qkv_passA_mm, 0, 2), mk(qkv_passA_ve, 0, 2),
                         mk_ln(0, 2, None), mk(qkv_passA_mm, 0, 3),
                         seq(mk(v_pair, 4), mk(v_pair, 5)),
                         seq(mk(qkv_passA_ve, 0, 3), mk_ln(0, 3, None)),
                         mk(v_pair, 6), mk(v_pair, 7)]),
            (0, 1, "B", [mk(qkv_passA_mm, 1, 0), mk(qkv_passA_ve, 1, 0),
                         mk(qkv_passA_mm, 1, 1), mk(qkv_passA_ve, 1, 1)]),
            (0, 2, "B", [mk(qkv_passA_mm, 1, 2), mk(qkv_passA_ve, 1, 2),
                         mk(qkv_passA_mm, 1, 3), mk(qkv_passA_ve, 1, 3),
                         mk_ln(1, 0, 1), mk_ft(1, 2)]),
            (0, 3, "B", [mk_ft(1, 3)]),
            (1, 0, "B", [mk(qkv_passA_mm, 2, 0), mk(qkv_passA_ve, 2, 0),
                         mk(qkv_passA_mm, 2, 1), mk(qkv_passA_ve, 2, 1)]),
            (1, 1, "B", [mk(qkv_passA_mm, 2, 2), mk(qkv_passA_ve, 2, 2),
                         mk(qkv_passA_mm, 2, 3), mk(qkv_passA_ve, 2, 3)]),
            (1, 2, "B", [mk_ln(2, 0, 1), mk_ft(2, 2), mk_ft(2, 3)]),
            (1, 3, "B", []),
            (2, 0, "B", []),
            (2, 1, "B", [mk(proj_tt, 0), mk(proj_tt, 1), mk(proj_tt, 2),
                         mk(proj_tt, 3)]),
            (2, 2, "B", [mk(proj_tt, 4), mk(proj_tt, 5), mk(proj_tt, 6),
                         mk(proj_tt, 7)]),
            (2, 3, "B", [mk(proj_tt, 8), mk(proj_tt, 9), mk(proj_tt, 10),
                         mk(proj_tt, 11)]),
        ]
        NS = len(STAGES)

        def S_of(si, g):
            h, qt, _, _ = STAGES[si]
            s_ps = (sA if g % 2 == 0 else sB).tile([128, 1024], F32, tag="s")
            qs = slice(qt * 512, (qt + 1) * 512)
            for j in range(2):
                kb = 2 * g + j
                mm(s_ps[:, j * 512:(j + 1) * 512],
                   kT(h)[:, kb * 128:(kb + 1) * 128], qT(h)[:, qs],
                   True, True)
            return s_ps

        def mk_pv(h, o_t, px, g):
            def run():
                for j in range(2):
                    kb = 2 * g + j
                    for qc in range(4):
                        mm(o_t[:, qc * 65:qc * 65 + 65],
                           px[:, j * 512 + qc * 128:j * 512 + (qc + 1) * 128],
                           v3i[:, (h * KB + kb) * 65:(h * KB + kb + 1) * 65],
                           kb == 0, kb == KB - 1)
            return run

        prev = None        # (h, qt, o_t) of previous stage, epilogue pending
        pv_pending = None  # deferred PV of the previous group
        s_cur = S_of(0, 0)
        for si in range(NS):
            h, qt, tmpl, items = STAGES[si]
            slots = list(items)
            if prev is not None:
                ph, pqt, po_t = prev
                box = {}

                def mk_ea(ph=ph, pqt=pqt, po_t=po_t, box=box):
                    def run():
                        box["o_n"] = epi_a(ph, pqt, po_t)
                    return run

                def mk_eb(ph=ph, pqt=pqt, po_t=po_t, box=box):
                    return lambda: epi_b(ph, pqt, po_t, box["o_n"])

                slots = [mk_ea(), mk_eb()] + slots
            assert len(slots) <= 8, (si, len(slots))
            o_t = (oA if si % 2 == 0 else oB).tile([128, 512], F32, tag="o")
            for g in range(8):
                px = pe.tile([128, 1024], BF16, tag="pexp")
                nc.scalar.activation(px[:], s_cur[:], AF.Exp,
                                     bias=0.0, scale=0.125)
                if tmpl == "A":
                    if pv_pending is not None:
                        pv_pending()
                        pv_pending = None
                    if g < len(slots):
                        slots[g]()
                    if g < 7:
                        s_cur = S_of(si, g + 1)
                    elif si + 1 < NS:
                        s_cur = S_of(si + 1, 0)
                    mk_pv(h, o_t, px, g)()
                else:
                    if g < 7:
                        s_cur = S_of(si, g + 1)
                    elif si + 1 < NS:
                        s_cur = S_of(si + 1, 0)
                    if pv_pending is not None:
                        pv_pending()
                    pv_pending = mk_pv(h, o_t, px, g)
                    if g < len(slots):
                        slots[g]()
            prev = (h, qt, o_t)

        if pv_pending is not None:
            pv_pending()  # last PV group
        # tail: last epilogue + last projection q-tile
        ph, pqt, po_t = prev
        o_n = epi_a(ph, pqt, po_t)
        epi_b(ph, pqt, po_t, o_n, tail=True)
        for tt in range(12, 16):
            proj_tt(tt, tail=True)

    if split_waits:
        _split_waits(nc)
    return nc


def _split_waits(nc):
    """This walrus build lowers at most one sync-wait per instruction (the
    matmul LDW struct rejects 2+). Move excess waits onto NoOps inserted
    just before, on the same engine queue — queues are in-order, so the
    constraint is preserved exactly."""
    k = 0
    for fn in nc.m.functions:
        for bb in fn.blocks:
            il = bb.instructions
            idx = 0
            while idx < len(il):
                inst = il[idx]
                si = inst.sync_info
                eng = getattr(inst, "engine", None)
                if (si is not None and len(si.on_wait) > 1
                        and eng is not None
                        and str(eng) != "EngineType.Unassigned"):
                    waits = list(si.on_wait)
                    inst.sync_info = mybir.SyncInfo(
                        on_wait=[waits[-1]], on_update=list(si.on_update))
                    for w in waits[:-1]:
                        nop = mybir.InstNoOp(
                            name=f"I-waitnop-{k}", engine=eng, ins=[], outs=[],
                            sync_info=mybir.SyncInfo(on_wait=[w], on_update=[]))
                        k += 1
                        il.insert(idx, nop)
                        idx += 1
                idx += 1


def _bf16(a):
    return np.asarray(a, dtype=np.float32).astype(mybir.dt.np(BF16))


def _prep_core_inputs(core, x, rope_cos, rope_sin, qkv_kernel, qkv_bias,
                      proj_kernel, proj_bias, q_norm_w, k_norm_w):
    b = core // 4
    heads = [3 * (core % 4) + i for i in range(HP)]

    wq = qkv_kernel.reshape(C, 3, H, HD)
    bq = qkv_bias.reshape(3, H, HD)

    xT = np.ascontiguousarray(x[b].T, dtype=np.float32)

    wqk2 = np.empty((128, CCH * HP * 128), np.float32)
    wv2 = np.empty((128, CCH * HP * 64), np.float32)
    for c in range(CCH):
        rows = slice(c * 128, (c + 1) * 128)
        for i, h in enumerate(heads):
            base = c * HP * 128 + i * 128
            wqk2[:, base:base + 64] = wq[rows, 0, h][:, PERM]
            wqk2[:, base + 64:base + 128] = wq[rows, 1, h][:, PERM]
            wv2[:, c * HP * 64 + i * 64:c * HP * 64 + (i + 1) * 64] = \
                wq[rows, 2, h]

    bqkbv = np.empty((1, HP * 192), np.float32)
    for i, h in enumerate(heads):
        bqkbv[0, i * 128:i * 128 + 64] = bq[0, h, PERM]
        bqkbv[0, i * 128 + 64:(i + 1) * 128] = bq[1, h, PERM]
        bqkbv[0, HP * 128 + i * 64:HP * 128 + (i + 1) * 64] = bq[2, h]

    cosT = rope_cos.T  # (HD, N)
    sinT = rope_sin.T
    cos2w = np.empty((128, N), np.float32)
    sinSw = np.empty((128, N), np.float32)
    cos2w[0:64] = cosT[PERM] * q_norm_w[PERM][:, None]
    cos2w[64:128] = cosT[PERM] * k_norm_w[PERM][:, None]
    sinSw[0:64] = SIGN[:, None] * sinT[PERM] * q_norm_w[PERM][:, None]
    sinSw[64:128] = SIGN[:, None] * sinT[PERM] * k_norm_w[PERM][:, None]

    rows = np.concatenate([np.arange(h * HD, (h + 1) * HD) for h in heads])
    wp = np.ascontiguousarray(proj_kernel[rows, :], dtype=np.float32)

    identd = np.eye(128, dtype=np.float32)

    return {"xT": _bf16(xT), "wqk2": _bf16(wqk2), "wv2": _bf16(wv2),
            "bqkbv": _bf16(bqkbv),
            "cos2w": _bf16(cos2w), "sinSw": _bf16(sinSw),
            "wp": _bf16(wp), "identd": identd}


def kernel(x, rope_cos, rope_sin, qkv_kernel, qkv_bias, proj_kernel,
           proj_bias, q_norm_w, k_norm_w, _trace=False):
    args = [np.asarray(a, dtype=np.float32) for a in
            (x, rope_cos, rope_sin, qkv_kernel, qkv_bias, proj_kernel,
             proj_bias, q_norm_w, k_norm_w)]
    in_maps = [_prep_core_inputs(c, *args) for c in range(NCORES)]

    if "nc" not in _NC_CACHE:
        _NC_CACHE["nc"] = build_nc()
    nc = _NC_CACHE["nc"]

    res = run_bass_kernel_spmd(nc, in_maps, core_ids=list(range(NCORES)),
                               trace=_trace)
    parts = [np.asarray(res.results[c]["out"], dtype=np.float32)
             for c in range(NCORES)]
    out = np.empty((B, N, C), np.float32)
    pb = np.asarray(proj_bias, dtype=np.float32)
    for b in range(B):
        out[b] = parts[4 * b] + parts[4 * b + 1] + parts[4 * b + 2] + parts[4 * b + 3] + pb
    if _trace:
        kernel.last_results = res
    return out
